# revision 45
# baseline (speedup 1.0000x reference)
"""DEMA (Holt double exponential smoothing) Trainium2 Bass kernel.

Math: the recurrence
    h_t = A h_{t-1} + v * x_t,  A = [[1-a, 1-a], [-ab, 1-ab]],  v = [a, ab]
has spectral radius sqrt(1-a) ~ 0.837, so the impulse response
w_j = e1^T A^j v decays below fp16 noise by j ~ 64.  s_t is then (for
fp16 purposes) an exact causal convolution with a <=256-tap kernel,
evaluated as a banded-triangular matmul over time chunks of 128:

    s_chunk[i, n] = sum_k Wcur[k, i] x_cur[k, n] + sum_k Wprev[k, i] x_prev[k, n]

with time-within-chunk on the partition (contraction) axis and the 512
fused (batch, channel) sequences on the moving free axis.  Chunk 0 uses
a modified Wcur (W0) that absorbs the s0 = x0, b0 = x1 - x0 initial
condition.  No cross-chunk serial dependency remains, so all 63 matmuls
per core are independent and pipeline freely.

The problem is HBM-bandwidth bound (~300-400 GB/s combined per core,
core-dependent), so the whole data path runs in fp16 (total 8.5 MB HBM
traffic per core; rel err ~3e-4, far inside the 2e-2 gate) and the
device layout is chunk-blocked [128, NCH, NF]: partition p =
time-within-chunk, free axis = (chunk, sequence).  The host
pre/post-permutes, so every device DMA is a fully contiguous multi-KB
read/write per partition -- minimal descriptor count.

DMA schedule (the measured constraint is ~200 GB/s per HWDGE ring, so
both rings must stay loaded the whole time):
 * one fused weight-slab DMA goes first on the Sync ring (a late
   weight stalls the PE >3.4 us and re-throttles the HAM clock gate);
 * the first four input groups go back-to-back on the Scalar ring
   (each dispatch occupies its engine ~0.65 us, so stacking weights +
   inputs on one ring delays the early chunks by microseconds);
 * remaining input groups and the output groups alternate across both
   rings; all input dispatches are emitted up front and are wait-free
   (xin bufs cover every group), so no output dispatch's semaphore
   wait can head-of-line-block an input behind it in an engine FIFO.

PSUM tiles hold two chunks (2 banks); one Vector/Scalar copy evacuates
both (FD=1024) into fp16 output tiles, halving the copy instruction
count; copies alternate Vector/Scalar so neither engine is the
bottleneck.  No warmup matmuls: the real matmul stream is dense enough
to un-throttle HAM by itself, and a warmup burst just delays the first
real matmuls (PE order is strictly FIFO).

Sharding: data-parallel on batch B=64 across 8 cores (8 batches/core).
Measured (NTFF span, worst core of 8): ~46 us; ~15 us of that is the
fixed NEFF preamble (engine barriers + TENSOR_LOADs) + postamble
(blanket 256-semaphore reset), which any kernel pays.
"""

import sys

import numpy as np

if "/opt/trn_rl_repo" not in sys.path:
    sys.path.insert(0, "/opt/trn_rl_repo")

import concourse.mybir as mybir  # noqa: E402
from concourse import bacc, bass_utils  # noqa: E402
from concourse.tile import TileContext  # noqa: E402

ALPHA, BETA = 0.3, 0.1
B, T, C = 64, 4096, 64
NCORES = 8
BL = B // NCORES          # local batch per core
L = 128                   # chunk length (time steps on partitions)
NCH = T // L              # 32 chunks
NF = BL * C               # 512 fused sequences on the moving free axis

MM_DT = mybir.dt.float16
MM_NP = np.float16

# A single HWDGE queue sustains only ~200 GB/s of HBM reads (latency
# bound), so input groups are striped across BOTH HWDGE rings (Sync and
# Scalar).  All input dispatches are emitted up front and -- with enough
# xin buffers -- carry no waits, so neither engine's FIFO ever blocks an
# input behind an output dispatch.  Output groups alternate between the
# rings after the inputs.
IGROUPS = [2, 2, 4, 4, 4, 4, 4, 4, 4]
# The first four input groups go consecutively on the Scalar ring so the
# early input stream is dense (a DMA dispatch occupies its engine
# ~0.65 us, so stacking weights+inputs on one ring delays early chunks
# by multiple microseconds).  Sync carries the weight slab + later
# groups.
ISYNC = {4, 6, 8}         # input group indices dispatched on Sync
OGROUPS = [2, 2, 4, 4, 4, 4, 4, 4, 2, 2]


def _make_weights():
    A = np.array([[1 - ALPHA, 1 - ALPHA], [-ALPHA * BETA, 1 - ALPHA * BETA]],
                 dtype=np.float64)
    v = np.array([ALPHA, ALPHA * BETA], dtype=np.float64)
    w = np.zeros(2 * L, dtype=np.float64)
    e1A = np.zeros((2 * L, 2), dtype=np.float64)
    w[0] = ALPHA
    e1A[0] = [1.0, 0.0]
    Aj = A.copy()
    for j in range(1, 2 * L):
        w[j] = Aj[0] @ v
        e1A[j] = Aj[0]
        Aj = Aj @ A
    k = np.arange(L)[:, None]
    i = np.arange(L)[None, :]
    Wcur = np.where(i >= k, w[np.clip(i - k, 0, None)], 0.0)
    Wprev = w[128 + i - k]
    W0 = Wcur.copy()
    W0[0, 0], W0[1, 0] = 1.0, 0.0
    ii = np.arange(1, L)
    W0[0, 1:] = e1A[ii] @ [1.0, -1.0]
    W0[1, 1:] = e1A[ii] @ [0.0, 1.0] + w[ii - 1]
    return np.ascontiguousarray(
        np.concatenate([W0, Wcur, Wprev], axis=1), dtype=MM_NP)


def _build_program():
    assert sum(IGROUPS) == NCH and sum(OGROUPS) == NCH
    nc = bacc.Bacc("TRN2", target_bir_lowering=False)
    # Chunk-blocked layout: [partition = time-within-chunk, chunk, seq].
    x = nc.dram_tensor("x", [L, NCH, NF], MM_DT, kind="ExternalInput")
    y = nc.dram_tensor("y", [L, NCH, NF], MM_DT, kind="ExternalOutput")
    # All three weight matrices ride in one slab -> one DMA dispatch.
    wall_d = nc.dram_tensor("wall", [L, 3 * L], MM_DT, kind="ExternalInput")
    with TileContext(nc) as tc:
        with (
            tc.tile_pool(name="const", bufs=1) as cpool,
            tc.tile_pool(name="xin", bufs=9) as xpool,
            tc.tile_pool(name="psum", bufs=4, space="PSUM") as ppool,
            tc.tile_pool(name="yout", bufs=4) as opool,
        ):
            wall = cpool.tile([L, 3 * L], MM_DT, tag="wall")
            # Weight slab first on the (otherwise idle-at-start) Sync ring:
            # a late weight stalls the PE >3.4 us and re-throttles HAM.
            nc.sync.dma_start(wall[:], wall_d[:, :])
            # HAM warmup with ZERO DMA dependencies: a memset junk tile is
            # ready at preamble exit (~6.5 us), so these throwaway matmuls
            # start ~3 us before the first input chunk lands and the HAM
            # clock gate opens (1.2 -> 2.4 GHz) just as real matmuls
            # arrive, instead of 3.4 us into them.  They finish before the
            # first input data is ready, so they delay nothing (PE FIFO).
            junk = cpool.tile([L, NF], MM_DT, tag="junk")
            nc.vector.memset(junk[:], 0.0)
            pwarm = ppool.tile([128, 2, NF], mybir.dt.float32,
                               name="pwarm", tag="p")
            for i in range(5):
                nc.tensor.matmul(pwarm[:, i % 2, :], junk[:, :L], junk[:],
                                 start=True, stop=True)
            w0 = wall[:, 0 * L:1 * L]
            wcur = wall[:, 1 * L:2 * L]
            wprev = wall[:, 2 * L:3 * L]
            # Emit every input-group dispatch up front: with xin bufs
            # covering all groups they carry no waits, so both HWDGE rings
            # stream input from t=0 with no head-of-line blocking.
            xgs = []
            off = 0
            for gidx, gi in enumerate(IGROUPS):
                xg = xpool.tile([128, gi, NF], MM_DT,
                                name=f"xg{off}", tag="xg",
                                padded_shape=[128, max(IGROUPS), NF])
                ieng = nc.sync if gidx in ISYNC else nc.scalar
                ieng.dma_start(xg[:], x[:, off:off + gi, :])
                xgs.append((off, gi, xg))
                off += gi
            xprev = None
            ot = None
            ps = None
            og = list(OGROUPS)
            ostart = ioff = ooff = 0
            ncopy = nog = 0
            gi_iter = iter(xgs)
            xg = None
            iend = 0
            for c in range(NCH):
                if c == iend:
                    ioff, glen, xg = next(gi_iter)
                    iend = ioff + glen
                xt = xg[:, c - ioff, :]
                if c % 2 == 0:
                    ps = ppool.tile([128, 2, NF], mybir.dt.float32,
                                    name=f"p{c}", tag="p")
                pv = ps[:, c % 2, :]
                nc.tensor.matmul(pv, w0 if c == 0 else wcur, xt,
                                 start=True, stop=(c == 0))
                if c > 0:
                    nc.tensor.matmul(pv, wprev, xprev,
                                     start=False, stop=True)
                if c == ostart:
                    go = og.pop(0)
                    ot = opool.tile([128, go, NF], MM_DT,
                                    name=f"yg{c}", tag="yg",
                                    padded_shape=[128, max(OGROUPS), NF])
                    ooff = ostart
                    ostart += go
                if c % 2 == 1:
                    # Evacuate both PSUM banks of the pair in one op,
                    # alternating Vector/Scalar per pair so each output
                    # group's dispatch waits on both engines' most recent
                    # short copy rather than two serial copies on one.
                    dst = ot[:, c - 1 - ooff:c + 1 - ooff, :]
                    if ncopy % 2 == 1:
                        nc.scalar.copy(dst, ps[:, :, :])
                    else:
                        nc.vector.tensor_copy(dst, ps[:, :, :])
                    ncopy += 1
                if c == ostart - 1:
                    oeng = nc.sync if nog % 2 == 0 else nc.scalar
                    oeng.dma_start(y[:, ooff:ostart, :], ot[:, :, :])
                    nog += 1
                xprev = xt
    nc.compile()
    return nc


_NC = None


def _in_maps(x: np.ndarray):
    """x: full [B, T, C] fp32 -> per-core chunk-blocked fp16 inputs."""
    Wall = _make_weights()
    x16 = x.astype(np.float16)
    maps = []
    for r in range(NCORES):
        xl = x16[r * BL:(r + 1) * BL]                     # [BL, T, C]
        xt = xl.transpose(1, 0, 2).reshape(T, NF)         # time-major
        # [T, NF] -> [NCH, L, NF] -> [L, NCH, NF] (chunk-blocked)
        xp = np.ascontiguousarray(
            xt.reshape(NCH, L, NF).transpose(1, 0, 2))
        maps.append({"x": xp, "wall": Wall})
    return maps


def _gather(results) -> np.ndarray:
    outs = []
    for r in range(NCORES):
        yp = results[r]["y"].astype(np.float32)           # [L, NCH, NF]
        yt = yp.transpose(1, 0, 2).reshape(T, BL, C)      # time-major
        outs.append(yt.transpose(1, 0, 2))                # [BL, T, C]
    return np.ascontiguousarray(np.concatenate(outs, axis=0))


def kernel(x: np.ndarray) -> np.ndarray:
    global _NC
    if _NC is None:
        _NC = _build_program()
    x = np.ascontiguousarray(x, dtype=np.float32)
    res = bass_utils.run_bass_kernel_spmd(_NC, _in_maps(x),
                                          core_ids=list(range(NCORES)))
    return _gather(res.results)


# revision 46
# speedup vs baseline: 1.0001x; 1.0001x over previous
"""DEMA (Holt double exponential smoothing) Trainium2 Bass kernel.

Math: the recurrence
    h_t = A h_{t-1} + v * x_t,  A = [[1-a, 1-a], [-ab, 1-ab]],  v = [a, ab]
has spectral radius sqrt(1-a) ~ 0.837, so the impulse response
w_j = e1^T A^j v decays below fp16 noise by j ~ 64.  s_t is then (for
fp16 purposes) an exact causal convolution with a <=256-tap kernel,
evaluated as a banded-triangular matmul over time chunks of 128:

    s_chunk[i, n] = sum_k Wcur[k, i] x_cur[k, n] + sum_k Wprev[k, i] x_prev[k, n]

with time-within-chunk on the partition (contraction) axis and the 512
fused (batch, channel) sequences on the moving free axis.  Chunk 0 uses
a modified Wcur (W0) that absorbs the s0 = x0, b0 = x1 - x0 initial
condition.  No cross-chunk serial dependency remains, so all 63 matmuls
per core are independent and pipeline freely.

The problem is HBM-bandwidth bound (~300-400 GB/s combined per core,
core-dependent), so the whole data path runs in fp16 (total 8.5 MB HBM
traffic per core; rel err ~3e-4, far inside the 2e-2 gate) and the
device layout is chunk-blocked [128, NCH, NF]: partition p =
time-within-chunk, free axis = (chunk, sequence).  The host
pre/post-permutes, so every device DMA is a fully contiguous multi-KB
read/write per partition -- minimal descriptor count.

DMA schedule (the measured constraint is ~200 GB/s per HWDGE ring, so
both rings must stay loaded the whole time):
 * one fused weight-slab DMA goes first on the Sync ring (a late
   weight stalls the PE >3.4 us and re-throttles the HAM clock gate);
 * the first four input groups go back-to-back on the Scalar ring
   (each dispatch occupies its engine ~0.65 us, so stacking weights +
   inputs on one ring delays the early chunks by microseconds);
 * remaining input groups and the output groups alternate across both
   rings; all input dispatches are emitted up front and are wait-free
   (xin bufs cover every group), so no output dispatch's semaphore
   wait can head-of-line-block an input behind it in an engine FIFO.

PSUM tiles hold two chunks (2 banks); one Vector/Scalar copy evacuates
both (FD=1024) into fp16 output tiles, halving the copy instruction
count; copies alternate Vector/Scalar so neither engine is the
bottleneck.  No warmup matmuls: the real matmul stream is dense enough
to un-throttle HAM by itself, and a warmup burst just delays the first
real matmuls (PE order is strictly FIFO).

Sharding: data-parallel on batch B=64 across 8 cores (8 batches/core).
Measured (NTFF span, worst core of 8): ~46 us; ~15 us of that is the
fixed NEFF preamble (engine barriers + TENSOR_LOADs) + postamble
(blanket 256-semaphore reset), which any kernel pays.
"""

import sys

import numpy as np

if "/opt/trn_rl_repo" not in sys.path:
    sys.path.insert(0, "/opt/trn_rl_repo")

import concourse.mybir as mybir  # noqa: E402
from concourse import bacc, bass_utils  # noqa: E402
from concourse.tile import TileContext  # noqa: E402

ALPHA, BETA = 0.3, 0.1
B, T, C = 64, 4096, 64
NCORES = 8
BL = B // NCORES          # local batch per core
L = 128                   # chunk length (time steps on partitions)
NCH = T // L              # 32 chunks
NF = BL * C               # 512 fused sequences on the moving free axis

MM_DT = mybir.dt.float16
MM_NP = np.float16

# A single HWDGE queue sustains only ~200 GB/s of HBM reads (latency
# bound), so input groups are striped across BOTH HWDGE rings (Sync and
# Scalar).  All input dispatches are emitted up front and -- with enough
# xin buffers -- carry no waits, so neither engine's FIFO ever blocks an
# input behind an output dispatch.  Output groups alternate between the
# rings after the inputs.
IGROUPS = [2, 2, 4, 4, 4, 4, 4, 4, 4]
# The first four input groups go consecutively on the Scalar ring so the
# early input stream is dense (a DMA dispatch occupies its engine
# ~0.65 us, so stacking weights+inputs on one ring delays early chunks
# by multiple microseconds).  Sync carries the weight slab + later
# groups.
ISYNC = {4, 6, 8}         # input group indices dispatched on Sync
OGROUPS = [2, 2, 4, 4, 4, 4, 4, 4, 2, 2]


def _make_weights():
    A = np.array([[1 - ALPHA, 1 - ALPHA], [-ALPHA * BETA, 1 - ALPHA * BETA]],
                 dtype=np.float64)
    v = np.array([ALPHA, ALPHA * BETA], dtype=np.float64)
    w = np.zeros(2 * L, dtype=np.float64)
    e1A = np.zeros((2 * L, 2), dtype=np.float64)
    w[0] = ALPHA
    e1A[0] = [1.0, 0.0]
    Aj = A.copy()
    for j in range(1, 2 * L):
        w[j] = Aj[0] @ v
        e1A[j] = Aj[0]
        Aj = Aj @ A
    k = np.arange(L)[:, None]
    i = np.arange(L)[None, :]
    Wcur = np.where(i >= k, w[np.clip(i - k, 0, None)], 0.0)
    Wprev = w[128 + i - k]
    W0 = Wcur.copy()
    W0[0, 0], W0[1, 0] = 1.0, 0.0
    ii = np.arange(1, L)
    W0[0, 1:] = e1A[ii] @ [1.0, -1.0]
    W0[1, 1:] = e1A[ii] @ [0.0, 1.0] + w[ii - 1]
    return np.ascontiguousarray(
        np.concatenate([W0, Wcur, Wprev], axis=1), dtype=MM_NP)


def _build_program():
    assert sum(IGROUPS) == NCH and sum(OGROUPS) == NCH
    nc = bacc.Bacc("TRN2", target_bir_lowering=False)
    # Chunk-blocked layout: [partition = time-within-chunk, chunk, seq].
    x = nc.dram_tensor("x", [L, NCH, NF], MM_DT, kind="ExternalInput")
    y = nc.dram_tensor("y", [L, NCH, NF], MM_DT, kind="ExternalOutput")
    # All three weight matrices ride in one slab -> one DMA dispatch.
    wall_d = nc.dram_tensor("wall", [L, 3 * L], MM_DT, kind="ExternalInput")
    with TileContext(nc) as tc:
        with (
            tc.tile_pool(name="const", bufs=1) as cpool,
            tc.tile_pool(name="xin", bufs=9) as xpool,
            tc.tile_pool(name="psum", bufs=4, space="PSUM") as ppool,
            tc.tile_pool(name="yout", bufs=4) as opool,
        ):
            wall = cpool.tile([L, 3 * L], MM_DT, tag="wall")
            # Weight slab first on the (otherwise idle-at-start) Sync ring:
            # a late weight stalls the PE >3.4 us and re-throttles HAM.
            nc.sync.dma_start(wall[:], wall_d[:, :])
            # (No HAM warmup burst: measured, a dependency-free warmup
            # cannot accumulate the 3.4 us continuous-busy SHORT window
            # before real data arrives, and the early real matmuls are
            # input-paced with ~1.8 us gaps that dilute the window anyway
            # -- the PE warms at ~17 us in either case, and the warmup
            # only adds instructions.)
            w0 = wall[:, 0 * L:1 * L]
            wcur = wall[:, 1 * L:2 * L]
            wprev = wall[:, 2 * L:3 * L]
            # Emit every input-group dispatch up front: with xin bufs
            # covering all groups they carry no waits, so both HWDGE rings
            # stream input from t=0 with no head-of-line blocking.
            xgs = []
            off = 0
            for gidx, gi in enumerate(IGROUPS):
                xg = xpool.tile([128, gi, NF], MM_DT,
                                name=f"xg{off}", tag="xg",
                                padded_shape=[128, max(IGROUPS), NF])
                ieng = nc.sync if gidx in ISYNC else nc.scalar
                ieng.dma_start(xg[:], x[:, off:off + gi, :])
                xgs.append((off, gi, xg))
                off += gi
            xprev = None
            ot = None
            ps = None
            og = list(OGROUPS)
            ostart = ioff = ooff = 0
            ncopy = nog = 0
            gi_iter = iter(xgs)
            xg = None
            iend = 0
            for c in range(NCH):
                if c == iend:
                    ioff, glen, xg = next(gi_iter)
                    iend = ioff + glen
                xt = xg[:, c - ioff, :]
                if c % 2 == 0:
                    ps = ppool.tile([128, 2, NF], mybir.dt.float32,
                                    name=f"p{c}", tag="p")
                pv = ps[:, c % 2, :]
                nc.tensor.matmul(pv, w0 if c == 0 else wcur, xt,
                                 start=True, stop=(c == 0))
                if c > 0:
                    nc.tensor.matmul(pv, wprev, xprev,
                                     start=False, stop=True)
                if c == ostart:
                    go = og.pop(0)
                    ot = opool.tile([128, go, NF], MM_DT,
                                    name=f"yg{c}", tag="yg",
                                    padded_shape=[128, max(OGROUPS), NF])
                    ooff = ostart
                    ostart += go
                if c % 2 == 1:
                    # Evacuate both PSUM banks of the pair in one op,
                    # alternating Vector/Scalar per pair so each output
                    # group's dispatch waits on both engines' most recent
                    # short copy rather than two serial copies on one.
                    dst = ot[:, c - 1 - ooff:c + 1 - ooff, :]
                    if ncopy % 2 == 1:
                        nc.scalar.copy(dst, ps[:, :, :])
                    else:
                        nc.vector.tensor_copy(dst, ps[:, :, :])
                    ncopy += 1
                if c == ostart - 1:
                    oeng = nc.sync if nog % 2 == 0 else nc.scalar
                    oeng.dma_start(y[:, ooff:ostart, :], ot[:, :, :])
                    nog += 1
                xprev = xt
    nc.compile()
    return nc


_NC = None


def _in_maps(x: np.ndarray):
    """x: full [B, T, C] fp32 -> per-core chunk-blocked fp16 inputs."""
    Wall = _make_weights()
    x16 = x.astype(np.float16)
    maps = []
    for r in range(NCORES):
        xl = x16[r * BL:(r + 1) * BL]                     # [BL, T, C]
        xt = xl.transpose(1, 0, 2).reshape(T, NF)         # time-major
        # [T, NF] -> [NCH, L, NF] -> [L, NCH, NF] (chunk-blocked)
        xp = np.ascontiguousarray(
            xt.reshape(NCH, L, NF).transpose(1, 0, 2))
        maps.append({"x": xp, "wall": Wall})
    return maps


def _gather(results) -> np.ndarray:
    outs = []
    for r in range(NCORES):
        yp = results[r]["y"].astype(np.float32)           # [L, NCH, NF]
        yt = yp.transpose(1, 0, 2).reshape(T, BL, C)      # time-major
        outs.append(yt.transpose(1, 0, 2))                # [BL, T, C]
    return np.ascontiguousarray(np.concatenate(outs, axis=0))


def kernel(x: np.ndarray) -> np.ndarray:
    global _NC
    if _NC is None:
        _NC = _build_program()
    x = np.ascontiguousarray(x, dtype=np.float32)
    res = bass_utils.run_bass_kernel_spmd(_NC, _in_maps(x),
                                          core_ids=list(range(NCORES)))
    return _gather(res.results)
